# revision 29
# baseline (speedup 1.0000x reference)
"""AttentionPooling kernel for 8 Trainium2 NeuronCores.

Problem: nn.MultiheadAttention pooling with a single query token (q = x[:,0]),
k = v = x, key_padding_mask. B=32, S=4096, E=1024, H=16, D=64.

Key algebraic reformulation (avoids materializing K and V entirely):
    q_b            = x[b,0,:] @ Wq.T + bq                       (tiny)
    R_b[h,:]       = q_b[h,:] @ Wk[h*D:(h+1)*D, :] / sqrt(D)    (tiny)
    scores_b[h,s]  = R_b[h,:] . x[b,s,:]   (+ q.bk: cancels in softmax)
    p              = softmax(scores + maskbias)
    U_b[h,:]       = p_b[h,:] @ x_b                             (S-contraction)
    ctx_b[h*D+d]   = Wv[h*D+d,:] . U_b[h,:] + bv
    out            = ctx @ out_w.T + out_b
    attn_weights   = mean_h p

This turns 550 GFLOP of projections into ~17 GFLOP of skinny matmuls, making
the kernel memory-bound on streaming x (512 MB) once, data-parallel over batch
(4 batches per core). All heavy PE work runs in fp16 (fp32 PSUM accumulate);
max-subtracted softmax keeps exp() in fp16 range.
"""

import os
import sys

import numpy as np

for _p in ("/opt/trn_rl_repo", "/root/.axon_site/_ro/trn_rl_repo"):
    if os.path.isdir(_p) and _p not in sys.path:
        sys.path.insert(0, _p)

import concourse.bass as bass
import concourse.tile as tile
from concourse import mybir
from concourse.bass_utils import run_bass_kernel_spmd
from concourse.masks import make_identity

B, S, E, H = 32, 4096, 1024, 16
D = E // H            # 64
NCORES = 8
BL = B // NCORES      # 4 local batches per core
EC = E // 128         # 8 e-chunks
SC = S // 128         # 32 s-chunks
ST = S // 512         # 8 s-tiles
F16 = mybir.dt.float16
F32 = mybir.dt.float32

_CACHE: dict = {}


def _split_multiwaits(nc: bass.Bass) -> None:
    """The walrus build in this container accepts only ONE sync wait per
    instruction (setupSyncWait: "Too many sync wait commands").  Tile's
    scheduler freely attaches several.  Splitting is semantically neutral:
    engines consume their instruction stream in order, so hoisting all but
    one wait onto standalone EventSemaphore instructions immediately before
    the real instruction enforces the same happens-before edges."""
    import bass_rust

    f = nc.m.functions[0]
    for blk in f.blocks:
        il = blk.instructions
        out = []
        changed = False
        for inst in il:
            si = inst.sync_info
            waits = list(si.on_wait) if si is not None and si.on_wait else []
            if len(waits) > 1:
                for w in waits[:-1]:
                    evt = mybir.InstEventSemaphore(
                        name=nc.get_next_instruction_name(),
                        engine=inst.engine,
                        ins=[], outs=[],
                        sync_info=bass_rust.SyncInfo(on_wait=[w], on_update=[]),
                    )
                    out.append(evt)
                inst.sync_info = bass_rust.SyncInfo(
                    on_wait=[waits[-1]], on_update=list(si.on_update or []))
                changed = True
            out.append(inst)
        if changed:
            blk.instructions = out


def _build_bass() -> bass.Bass:
    nc = bass.Bass()

    x_in = nc.declare_dram_parameter("x_in", [BL, S, E], F32, isOutput=False)
    mb_in = nc.declare_dram_parameter("mb_in", [BL, H, S], F16, isOutput=False)
    wqt = nc.declare_dram_parameter("wqt", [E, E], F16, isOutput=False)   # Wq.T/8 [e,o]
    wk = nc.declare_dram_parameter("wk", [E, E], F16, isOutput=False)     # Wk [o,e]
    wvt = nc.declare_dram_parameter("wvt", [E, E], F16, isOutput=False)   # Wv.T [e,o]
    wot = nc.declare_dram_parameter("wot", [E, E], F16, isOutput=False)   # out_w.T [o,o2]
    bq8 = nc.declare_dram_parameter("bq8", [E], F32, isOutput=False)      # bq/8
    bv_d = nc.declare_dram_parameter("bv_d", [E], F32, isOutput=False)
    ob_d = nc.declare_dram_parameter("ob_d", [E], F32, isOutput=False)
    out_o = nc.declare_dram_parameter("out_o", [BL, E], F32, isOutput=True)
    aw_o = nc.declare_dram_parameter("aw_o", [BL, S], F32, isOutput=True)

    x_ap = x_in[:]
    mb_ap = mb_in[:]

    with tile.TileContext(nc) as tc:
        with (
            tc.tile_pool(name="consts", bufs=1) as consts,
            tc.tile_pool(name="xpool", bufs=6) as xpool,
            tc.tile_pool(name="xtpool", bufs=3) as xtpool,
            tc.tile_pool(name="wstream", bufs=2) as wstream,
            tc.tile_pool(name="wkpool", bufs=2) as wkpool,
            tc.tile_pool(name="mbpool", bufs=2) as mbpool,
            tc.tile_pool(name="ppool", bufs=1) as ppool,
            tc.tile_pool(name="smpool", bufs=1) as smpool,
            tc.tile_pool(name="ptpool", bufs=2) as ptpool,
            tc.tile_pool(name="awpool", bufs=2) as awpool,
            tc.tile_pool(name="ps_tf", bufs=3, space="PSUM") as ps_tf,
            tc.tile_pool(name="ps_t", bufs=1, space="PSUM") as ps_t,
            tc.tile_pool(name="ps_s", bufs=2, space="PSUM") as ps_s,
            tc.tile_pool(name="ps_u", bufs=2, space="PSUM") as ps_u,
        ):
            # ---------------- constants ----------------
            i128 = consts.tile([128, 128], F16, tag="i128")
            make_identity(nc, i128[:])
            i16 = consts.tile([16, 16], F16, tag="i16")
            make_identity(nc, i16[:])

            warm_n = [0]

            def warm(n):
                # Filler matmuls on the identity: keep the PE's HAM activity
                # window busy across DMA-paced stretches so real matmuls run
                # at 2.4 GHz instead of the cold 1.2 GHz default.
                warm_n[0] += 1
                junk = ps_t.tile([128, 512], F32, tag="pt",
                                 name=f"warm{warm_n[0]}")
                for _ in range(n):
                    nc.tensor.matmul(junk[:, :128], i128, i128,
                                     start=True, stop=True)

            warm(48)

            bq8_sb = consts.tile([128, EC], F32, tag="bq8")
            nc.sync.dma_start(out=bq8_sb, in_=bq8[:].rearrange("(c p) -> p c", p=128))
            bv_sb = consts.tile([128, EC], F32, tag="bv")
            nc.sync.dma_start(out=bv_sb, in_=bv_d[:].rearrange("(c p) -> p c", p=128))
            ob_rep = consts.tile([BL, E], F32, tag="ob")
            ob_ap = ob_d[:]
            ob_bc = bass.AP(tensor=ob_ap.tensor, offset=ob_ap.offset,
                            ap=[[0, BL]] + list(ob_ap.ap))
            nc.gpsimd.dma_start(out=ob_rep, in_=ob_bc)

            # ---------------- setup: q, R, R^T ----------------
            # x0 = x[:, 0, :] cast to fp16, then transposed to [e, b]
            x0_sb = consts.tile([BL, E], F16, tag="x0")
            nc.gpsimd.dma_start(out=x0_sb, in_=x_ap[:, 0, :])
            x0t = consts.tile([128, EC, BL], F16, tag="x0t")
            for ec in range(EC):
                ps = ps_t.tile([128, 512], F32, tag="pt")
                nc.tensor.matmul(
                    ps[:, :BL], x0_sb[:, ec * 128:(ec + 1) * 128], i16[:BL, :BL],
                    start=True, stop=True)
                nc.vector.tensor_copy(out=x0t[:, ec, :], in_=ps[:, :BL])

            # q^T[o, b] = (Wq.T/8)^T-chunks . x0^T  (+ bq/8)
            qt_sb = consts.tile([128, EC, BL], F16, tag="qt")
            for oc in range(EC):
                wqc = wstream.tile([128, EC, 128], F16, tag="wstream")
                nc.sync.dma_start(
                    out=wqc,
                    in_=wqt[:][:, oc * 128:(oc + 1) * 128].rearrange(
                        "(c p) m -> p c m", p=128))
                qps = ps_s.tile([128, 512], F32, tag="s")
                for ec in range(EC):
                    nc.tensor.matmul(
                        qps[:, :BL], wqc[:, ec, :], x0t[:, ec, :],
                        start=(ec == 0), stop=(ec == EC - 1))
                nc.vector.tensor_tensor(
                    qt_sb[:, oc, :], qps[:, :BL],
                    bq8_sb[:, oc:oc + 1].to_broadcast((128, BL)),
                    mybir.AluOpType.add)

            # qbd^T [o, b*16+h] block-diagonal-expanded q (zeros elsewhere)
            qbd = consts.tile([128, EC, 4 * H], F16, tag="qbd")
            nc.gpsimd.memset(qbd[:], 0.0)
            for c in range(EC):
                v = qbd[:, c, :].rearrange("p (b h) -> p b h", h=H)
                nc.vector.tensor_copy(out=v[0:64, :, 2 * c], in_=qt_sb[0:64, c, :])
                nc.vector.tensor_copy(out=v[64:128, :, 2 * c + 1], in_=qt_sb[64:128, c, :])

            # R[bh, e] = qbd^T.T @ Wk  -> [64, 1024]
            r_sb = consts.tile([4 * H, E], F16, tag="r")
            for half in range(2):
                rps = ps_s.tile([128, 512], F32, tag="s")
                for kc in range(EC):
                    wkc = wkpool.tile([128, E], F16, tag="wk")
                    nc.sync.dma_start(out=wkc, in_=wk[:][kc * 128:(kc + 1) * 128, :])
                    nc.tensor.matmul(
                        rps[:4 * H, :], qbd[:, kc, :],
                        wkc[:, half * 512:(half + 1) * 512],
                        start=(kc == 0), stop=(kc == EC - 1))
                nc.scalar.copy(
                    out=r_sb[:, half * 512:(half + 1) * 512], in_=rps[:4 * H, :])

            # R^T [e, bh]
            rt_sb = consts.tile([128, EC, 4 * H], F16, tag="rt")
            for ec in range(EC):
                ps = ps_t.tile([128, 512], F32, tag="pt")
                nc.tensor.matmul(
                    ps[:, :4 * H], r_sb[:, ec * 128:(ec + 1) * 128],
                    i128[:4 * H, :4 * H], start=True, stop=True)
                nc.vector.tensor_copy(out=rt_sb[:, ec, :], in_=ps[:, :4 * H])

            # U^T for all batches, filled in the main loop, consumed in the tail
            ut_all = consts.tile([128, BL, EC, H], F16, tag="ut")

            # ---------------- main loop over local batches ----------------
            # The x-transposes are issued as REGULAR matmuls against an
            # identity rhs (not transpose-mode): transpose-mode runs at the
            # un-HAM'd 1.2 GHz clock (~226 ns/block measured) while a warm
            # regular matmul streams the same block in ~55 ns and keeps the
            # PE's HAM clock at 8/8 for the surrounding matmuls.
            state: dict = {}

            def emit_xdma(b):
                # x arrives in QUARTER tiles (8 s-chunks each): finer slot
                # granularity lets batch b+1's stream begin while U(b) is
                # still consuming the early quarters of batch b.
                src = x_ap[b].rearrange("(sc p) e -> p sc e", p=128)
                xh = []
                for q in range(4):
                    xt_buf = xpool.tile([128, SC // 4, E], F16, tag="xh",
                                        name=f"xh_{b}_{q}")
                    for g in range(2):
                        lo = q * 8 + g * 4
                        nc.gpsimd.dma_start(
                            out=xt_buf[:, g * 4:(g + 1) * 4, :],
                            in_=src[:, lo:lo + 4, :])
                    xh.append(xt_buf)
                state[b] = {
                    "xh": xh,
                    "scores": smpool.tile([H, S], F32, tag="scores",
                                          name=f"scores{b}"),
                    "mx_all": smpool.tile([H, ST], F32, tag="mx_all",
                                          name=f"mxall{b}"),
                }

            def passA_tile(b, t):
                st = state[b]
                xbuf = st["xh"][t // 2]
                sc0 = (t % 2) * 4
                xt = xtpool.tile([128, EC, 512], F16, tag="xt", name=f"xt{b}_{t}")
                for ec in range(EC):
                    ps = ps_tf.tile([128, 512], F32, tag="ptf", name=f"tp{b}_{t}_{ec}")
                    for k in range(4):
                        nc.tensor.matmul(
                            ps[:, k * 128:(k + 1) * 128],
                            xbuf[:, sc0 + k, ec * 128:(ec + 1) * 128],
                            i128[:], start=True, stop=True)
                    if ec % 2 == 0:
                        nc.scalar.copy(out=xt[:, ec, :], in_=ps[:])
                    else:
                        nc.vector.tensor_copy(out=xt[:, ec, :], in_=ps[:])
                sps = ps_s.tile([128, 512], F32, tag="s", name=f"sps{b}_{t}")
                for ec in range(EC):
                    nc.tensor.matmul(
                        sps[:H, :], rt_sb[:, ec, b * H:(b + 1) * H], xt[:, ec, :],
                        start=(ec == 0), stop=(ec == EC - 1))
                mbt = mbpool.tile([H, 512], F16, tag="mb", name=f"mb{b}_{t}")
                nc.sync.dma_start(out=mbt, in_=mb_ap[b, :, t * 512:(t + 1) * 512])
                nc.vector.tensor_tensor(
                    st["scores"][:, t * 512:(t + 1) * 512], sps[:H, :], mbt,
                    mybir.AluOpType.add)
                nc.vector.reduce_max(
                    out=st["mx_all"][:, t:t + 1],
                    in_=st["scores"][:, t * 512:(t + 1) * 512],
                    axis=mybir.AxisListType.X)

            emit_xdma(0)
            wvt_sb = None
            wot_sb = None
            for b in range(BL):
                if b + 1 < BL:
                    emit_xdma(b + 1)
                else:
                    # prefetch the tail's Wv.T / Wout.T into x-stream slots
                    # (they free up as U(BL-2) consumes its quarters)
                    wvt_sb = xpool.tile([128, EC, E], F16, tag="xh",
                                        name="wvt_sb")
                    nc.sync.dma_start(
                        out=wvt_sb,
                        in_=wvt[:].rearrange("(c p) m -> p c m", p=128))
                    wot_sb = xpool.tile([128, EC, E], F16, tag="xh",
                                        name="wot_sb")
                    nc.sync.dma_start(
                        out=wot_sb,
                        in_=wot[:].rearrange("(c p) m -> p c m", p=128))
                st = state[b]
                for t in range(2 if b > 0 else 0, ST):
                    if b == 0:
                        warm(24)
                    passA_tile(b, t)
                xh = st["xh"]
                scores_sb = st["scores"]
                mx_all = st["mx_all"]

                # softmax: exp(score - rowmax), accumulate denominator
                p_sb = ppool.tile([H, S], F16, tag="p")
                mx = ppool.tile([H, 1], F32, tag="mx")
                nc.vector.reduce_max(out=mx, in_=mx_all, axis=mybir.AxisListType.X)
                neg_mx = ppool.tile([H, 1], F32, tag="neg_mx")
                nc.scalar.mul(out=neg_mx, in_=mx, mul=-1.0)
                den = ppool.tile([H, 1], F32, tag="den")
                nc.scalar.activation(
                    out=p_sb, in_=scores_sb,
                    func=mybir.ActivationFunctionType.Exp,
                    bias=neg_mx, accum_out=den)
                recip = ppool.tile([H, 1], F32, tag="recip")
                nc.vector.reciprocal(out=recip, in_=den)
                d16 = ppool.tile([H, H], F16, tag="d16")
                nc.vector.tensor_tensor(
                    d16, i16, recip.to_broadcast((H, H)), mybir.AluOpType.mult)
                recip16 = ppool.tile([H, 1], F16, tag="recip16")
                nc.scalar.mul(out=recip16, in_=recip, mul=1.0 / H)

                # keep the PE warm during this batch's softmax chain by
                # pulling the first two pass-A tiles of the next batch forward
                if b + 1 < BL:
                    passA_tile(b + 1, 0)
                    passA_tile(b + 1, 1)

                # p^T (normalized) [s, h] per s-chunk, all in one PSUM bank
                ptT = ptpool.tile([128, SC, H], F16, tag="ptT")
                pps = ps_t.tile([128, 512], F32, tag="pt")
                for sc in range(SC):
                    nc.tensor.matmul(
                        pps[:, sc * H:(sc + 1) * H],
                        p_sb[:, sc * 128:(sc + 1) * 128], d16,
                        start=True, stop=True)
                nc.vector.tensor_copy(out=ptT[:], in_=pps[:])

                # pass B: U[h, e] = p @ x, accumulated over all 32 s-chunks
                ups = [ps_u.tile([128, 512], F32, tag="u", name=f"u{j}") for j in range(2)]
                for sc in range(SC):
                    xbuf = xh[sc // (SC // 4)]
                    scl = sc % (SC // 4)
                    for j in range(2):
                        nc.tensor.matmul(
                            ups[j][:H, :], ptT[:, sc, :],
                            xbuf[:, scl, j * 512:(j + 1) * 512],
                            start=(sc == 0), stop=(sc == SC - 1))
                u_sb = ppool.tile([H, E], F16, tag="u")
                for j in range(2):
                    nc.scalar.copy(
                        out=u_sb[:, j * 512:(j + 1) * 512], in_=ups[j][:H, :])

                # attn_weights = (1/H) * ones @ p  -> [1, S] then DMA out
                for t in range(ST):
                    aps = ps_s.tile([128, 512], F32, tag="s")
                    nc.tensor.matmul(
                        aps[:1, :], recip16, p_sb[:, t * 512:(t + 1) * 512],
                        start=True, stop=True)
                    awt = awpool.tile([1, 512], F32, tag="aw")
                    nc.scalar.copy(out=awt, in_=aps[:1, :])
                    nc.sync.dma_start(
                        out=aw_o[:][b:b + 1, t * 512:(t + 1) * 512], in_=awt)

                # U^T [e, h] for this batch
                utps = ps_t.tile([128, 512], F32, tag="pt")
                for ec in range(EC):
                    nc.tensor.matmul(
                        utps[:, ec * H:(ec + 1) * H],
                        u_sb[:, ec * 128:(ec + 1) * 128], i16,
                        start=True, stop=True)
                nc.vector.tensor_copy(out=ut_all[:, b, :, :], in_=utps[:, :EC * H])

            # ---------------- tail: ctx and output projection ----------------
            # ctx^T[o, b] = sum_e Wv.T[e, o] * U^T[e, h(o)]  (head-blockdiagonal)
            ctxt_sb = consts.tile([128, EC, BL], F16, tag="ctxt")
            for oc in range(EC):
                cps = ps_s.tile([128, 512], F32, tag="s")
                for hh in range(2):
                    h = 2 * oc + hh
                    for ec in range(EC):
                        nc.tensor.matmul(
                            cps[hh * 64:(hh + 1) * 64, :BL],
                            wvt_sb[:, ec, oc * 128 + hh * 64:
                                   oc * 128 + (hh + 1) * 64],
                            ut_all[:, :, ec, h],
                            start=(ec == 0), stop=(ec == EC - 1))
                nc.vector.tensor_tensor(
                    ctxt_sb[:, oc, :], cps[:, :BL],
                    bv_sb[:, oc:oc + 1].to_broadcast((128, BL)),
                    mybir.AluOpType.add)

            # out^T[o2, b] = Wout.T^T-chunks @ ctx^T
            outt_sb = consts.tile([128, EC, BL], F16, tag="outt")
            for o2c in range(EC):
                ops_ = ps_s.tile([128, 512], F32, tag="s")
                for oc in range(EC):
                    nc.tensor.matmul(
                        ops_[:, :BL],
                        wot_sb[:, oc, o2c * 128:(o2c + 1) * 128],
                        ctxt_sb[:, oc, :],
                        start=(oc == 0), stop=(oc == EC - 1))
                nc.scalar.copy(out=outt_sb[:, o2c, :], in_=ops_[:, :BL])

            # transpose out^T back to [b, o2], add out_b, DMA out
            out_sb = consts.tile([BL, E], F32, tag="out_sb")
            for g in range(2):
                onps = ps_t.tile([128, 512], F32, tag="pt")
                for k in range(4):
                    c = g * 4 + k
                    nc.tensor.matmul(
                        onps[:BL, k * 128:(k + 1) * 128],
                        outt_sb[:, c, :], i128[:],
                        start=True, stop=True)
                nc.vector.tensor_tensor(
                    out_sb[:, g * 512:(g + 1) * 512], onps[:BL, :],
                    ob_rep[:, g * 512:(g + 1) * 512], mybir.AluOpType.add)
            nc.sync.dma_start(out=out_o[:], in_=out_sb)

    _split_multiwaits(nc)
    return nc


def _get_nc() -> bass.Bass:
    if "nc" not in _CACHE:
        _CACHE["nc"] = _build_bass()
    return _CACHE["nc"]


def _prep_inputs(x, mask, in_proj_w, in_proj_b, out_w, out_b):
    x = np.asarray(x, dtype=np.float32)
    mask = np.asarray(mask)
    in_proj_w = np.asarray(in_proj_w, dtype=np.float32)
    in_proj_b = np.asarray(in_proj_b, dtype=np.float32)
    out_w = np.asarray(out_w, dtype=np.float32)
    out_b = np.asarray(out_b, dtype=np.float32)

    Wq, Wk, Wv = in_proj_w[:E], in_proj_w[E:2 * E], in_proj_w[2 * E:]
    bq, bv = in_proj_b[:E], in_proj_b[2 * E:]
    sc = 1.0 / np.sqrt(D)

    wqt = np.ascontiguousarray((Wq.T * sc)).astype(np.float16)
    wk16 = np.ascontiguousarray(Wk).astype(np.float16)
    wvt = np.ascontiguousarray(Wv.T).astype(np.float16)
    wot = np.ascontiguousarray(out_w.T).astype(np.float16)
    bq8 = (bq * sc).astype(np.float32)
    bv32 = bv.astype(np.float32)
    ob32 = out_b.astype(np.float32)

    mb = np.where(mask, np.float16(-60000.0), np.float16(0.0)).astype(np.float16)
    mb_rep = np.ascontiguousarray(
        np.broadcast_to(mb[:, None, :], (B, H, S))).astype(np.float16)

    in_maps = []
    for c in range(NCORES):
        lo, hi = c * BL, (c + 1) * BL
        in_maps.append({
            "x_in": np.ascontiguousarray(x[lo:hi]),
            "mb_in": np.ascontiguousarray(mb_rep[lo:hi]),
            "wqt": wqt, "wk": wk16, "wvt": wvt, "wot": wot,
            "bq8": bq8, "bv_d": bv32, "ob_d": ob32,
        })
    return in_maps


def run(trace=False, **inputs):
    nc = _get_nc()
    in_maps = _prep_inputs(**inputs)
    res = run_bass_kernel_spmd(
        nc, in_maps, core_ids=list(range(NCORES)), trace=trace)
    out = np.concatenate([r["out_o"] for r in res.results], axis=0)
    aw = np.concatenate([r["aw_o"] for r in res.results], axis=0)
    out = out.astype(np.float32)
    aw = aw.astype(np.float32)[:, None, :]
    return (out, aw), res


def kernel(**inputs):
    (out, aw), _ = run(trace=False, **inputs)
    return out, aw


# revision 30
# speedup vs baseline: 1.0406x; 1.0406x over previous
"""AttentionPooling kernel for 8 Trainium2 NeuronCores.

Problem: nn.MultiheadAttention pooling with a single query token (q = x[:,0]),
k = v = x, key_padding_mask. B=32, S=4096, E=1024, H=16, D=64.

Key algebraic reformulation (avoids materializing K and V entirely):
    q_b            = x[b,0,:] @ Wq.T + bq                       (tiny)
    R_b[h,:]       = q_b[h,:] @ Wk[h*D:(h+1)*D, :] / sqrt(D)    (tiny)
    scores_b[h,s]  = R_b[h,:] . x[b,s,:]   (+ q.bk: cancels in softmax)
    p              = softmax(scores + maskbias)
    U_b[h,:]       = p_b[h,:] @ x_b                             (S-contraction)
    ctx_b[h*D+d]   = Wv[h*D+d,:] . U_b[h,:] + bv
    out            = ctx @ out_w.T + out_b
    attn_weights   = mean_h p

This turns 550 GFLOP of projections into ~17 GFLOP of skinny matmuls, making
the kernel memory-bound on streaming x (512 MB) once, data-parallel over batch
(4 batches per core). All heavy PE work runs in fp16 (fp32 PSUM accumulate);
max-subtracted softmax keeps exp() in fp16 range.
"""

import os
import sys

import numpy as np

for _p in ("/opt/trn_rl_repo", "/root/.axon_site/_ro/trn_rl_repo"):
    if os.path.isdir(_p) and _p not in sys.path:
        sys.path.insert(0, _p)

import concourse.bass as bass
import concourse.tile as tile
from concourse import mybir
from concourse.bass_utils import run_bass_kernel_spmd
from concourse.masks import make_identity

B, S, E, H = 32, 4096, 1024, 16
D = E // H            # 64
NCORES = 8
BL = B // NCORES      # 4 local batches per core
EC = E // 128         # 8 e-chunks
SC = S // 128         # 32 s-chunks
ST = S // 512         # 8 s-tiles
F16 = mybir.dt.float16
F32 = mybir.dt.float32

_CACHE: dict = {}


def _split_multiwaits(nc: bass.Bass) -> None:
    """The walrus build in this container accepts only ONE sync wait per
    instruction (setupSyncWait: "Too many sync wait commands").  Tile's
    scheduler freely attaches several.  Splitting is semantically neutral:
    engines consume their instruction stream in order, so hoisting all but
    one wait onto standalone EventSemaphore instructions immediately before
    the real instruction enforces the same happens-before edges."""
    import bass_rust

    f = nc.m.functions[0]
    for blk in f.blocks:
        il = blk.instructions
        out = []
        changed = False
        for inst in il:
            si = inst.sync_info
            waits = list(si.on_wait) if si is not None and si.on_wait else []
            if len(waits) > 1:
                for w in waits[:-1]:
                    evt = mybir.InstEventSemaphore(
                        name=nc.get_next_instruction_name(),
                        engine=inst.engine,
                        ins=[], outs=[],
                        sync_info=bass_rust.SyncInfo(on_wait=[w], on_update=[]),
                    )
                    out.append(evt)
                inst.sync_info = bass_rust.SyncInfo(
                    on_wait=[waits[-1]], on_update=list(si.on_update or []))
                changed = True
            out.append(inst)
        if changed:
            blk.instructions = out


def _build_bass() -> bass.Bass:
    nc = bass.Bass()

    x_in = nc.declare_dram_parameter("x_in", [BL, S, E], F32, isOutput=False)
    mb_in = nc.declare_dram_parameter("mb_in", [BL, H, S], F16, isOutput=False)
    wqt = nc.declare_dram_parameter("wqt", [E, E], F16, isOutput=False)   # Wq.T/8 [e,o]
    wk = nc.declare_dram_parameter("wk", [E, E], F16, isOutput=False)     # Wk [o,e]
    wvt = nc.declare_dram_parameter("wvt", [E, E], F16, isOutput=False)   # Wv.T [e,o]
    wot = nc.declare_dram_parameter("wot", [E, E], F16, isOutput=False)   # out_w.T [o,o2]
    bq8 = nc.declare_dram_parameter("bq8", [E], F32, isOutput=False)      # bq/8
    bv_d = nc.declare_dram_parameter("bv_d", [E], F32, isOutput=False)
    ob_d = nc.declare_dram_parameter("ob_d", [E], F32, isOutput=False)
    out_o = nc.declare_dram_parameter("out_o", [BL, E], F32, isOutput=True)
    aw_o = nc.declare_dram_parameter("aw_o", [BL, S], F32, isOutput=True)

    x_ap = x_in[:]
    mb_ap = mb_in[:]

    with tile.TileContext(nc) as tc:
        with (
            tc.tile_pool(name="consts", bufs=1) as consts,
            tc.tile_pool(name="xpool", bufs=6) as xpool,
            tc.tile_pool(name="xtpool", bufs=3) as xtpool,
            tc.tile_pool(name="wstream", bufs=2) as wstream,
            tc.tile_pool(name="wkpool", bufs=2) as wkpool,
            tc.tile_pool(name="mbpool", bufs=2) as mbpool,
            tc.tile_pool(name="ppool", bufs=1) as ppool,
            tc.tile_pool(name="smpool", bufs=1) as smpool,
            tc.tile_pool(name="ptpool", bufs=2) as ptpool,
            tc.tile_pool(name="awpool", bufs=2) as awpool,
            tc.tile_pool(name="ps_tf", bufs=3, space="PSUM") as ps_tf,
            tc.tile_pool(name="ps_t", bufs=1, space="PSUM") as ps_t,
            tc.tile_pool(name="ps_s", bufs=2, space="PSUM") as ps_s,
            tc.tile_pool(name="ps_u", bufs=2, space="PSUM") as ps_u,
        ):
            # ---------------- constants ----------------
            i128 = consts.tile([128, 128], F16, tag="i128")
            make_identity(nc, i128[:])
            i16 = consts.tile([16, 16], F16, tag="i16")
            make_identity(nc, i16[:])

            warm_n = [0]

            def warm(n):
                # Filler matmuls on the identity: keep the PE's HAM activity
                # window busy across DMA-paced stretches so real matmuls run
                # at 2.4 GHz instead of the cold 1.2 GHz default.
                warm_n[0] += 1
                junk = ps_t.tile([128, 512], F32, tag="pt",
                                 name=f"warm{warm_n[0]}")
                for _ in range(n):
                    nc.tensor.matmul(junk[:, :128], i128, i128,
                                     start=True, stop=True)

            warm(48)

            bq8_sb = consts.tile([128, EC], F32, tag="bq8")
            nc.sync.dma_start(out=bq8_sb, in_=bq8[:].rearrange("(c p) -> p c", p=128))
            bv_sb = consts.tile([128, EC], F32, tag="bv")
            nc.sync.dma_start(out=bv_sb, in_=bv_d[:].rearrange("(c p) -> p c", p=128))
            ob_rep = consts.tile([BL, E], F32, tag="ob")
            ob_ap = ob_d[:]
            ob_bc = bass.AP(tensor=ob_ap.tensor, offset=ob_ap.offset,
                            ap=[[0, BL]] + list(ob_ap.ap))
            nc.gpsimd.dma_start(out=ob_rep, in_=ob_bc)

            # ---------------- setup: q, R, R^T ----------------
            # x0 = x[:, 0, :] cast to fp16, then transposed to [e, b]
            x0_sb = consts.tile([BL, E], F16, tag="x0")
            nc.gpsimd.dma_start(out=x0_sb, in_=x_ap[:, 0, :])
            x0t = consts.tile([128, EC, BL], F16, tag="x0t")
            for ec in range(EC):
                ps = ps_t.tile([128, 512], F32, tag="pt")
                nc.tensor.matmul(
                    ps[:, :BL], x0_sb[:, ec * 128:(ec + 1) * 128], i16[:BL, :BL],
                    start=True, stop=True)
                nc.vector.tensor_copy(out=x0t[:, ec, :], in_=ps[:, :BL])

            # q^T[o, b] = (Wq.T/8)^T-chunks . x0^T  (+ bq/8)
            qt_sb = consts.tile([128, EC, BL], F16, tag="qt")
            for oc in range(EC):
                wqc = wstream.tile([128, EC, 128], F16, tag="wstream")
                nc.sync.dma_start(
                    out=wqc,
                    in_=wqt[:][:, oc * 128:(oc + 1) * 128].rearrange(
                        "(c p) m -> p c m", p=128))
                qps = ps_s.tile([128, 512], F32, tag="s")
                for ec in range(EC):
                    nc.tensor.matmul(
                        qps[:, :BL], wqc[:, ec, :], x0t[:, ec, :],
                        start=(ec == 0), stop=(ec == EC - 1))
                nc.vector.tensor_tensor(
                    qt_sb[:, oc, :], qps[:, :BL],
                    bq8_sb[:, oc:oc + 1].to_broadcast((128, BL)),
                    mybir.AluOpType.add)

            # qbd^T [o, b*16+h] block-diagonal-expanded q (zeros elsewhere)
            qbd = consts.tile([128, EC, 4 * H], F16, tag="qbd")
            nc.gpsimd.memset(qbd[:], 0.0)
            for c in range(EC):
                v = qbd[:, c, :].rearrange("p (b h) -> p b h", h=H)
                nc.vector.tensor_copy(out=v[0:64, :, 2 * c], in_=qt_sb[0:64, c, :])
                nc.vector.tensor_copy(out=v[64:128, :, 2 * c + 1], in_=qt_sb[64:128, c, :])

            # R[bh, e] = qbd^T.T @ Wk  -> [64, 1024]
            r_sb = consts.tile([4 * H, E], F16, tag="r")
            for half in range(2):
                rps = ps_s.tile([128, 512], F32, tag="s")
                for kc in range(EC):
                    wkc = wkpool.tile([128, E], F16, tag="wk")
                    nc.sync.dma_start(out=wkc, in_=wk[:][kc * 128:(kc + 1) * 128, :])
                    nc.tensor.matmul(
                        rps[:4 * H, :], qbd[:, kc, :],
                        wkc[:, half * 512:(half + 1) * 512],
                        start=(kc == 0), stop=(kc == EC - 1))
                nc.scalar.copy(
                    out=r_sb[:, half * 512:(half + 1) * 512], in_=rps[:4 * H, :])

            # R^T [e, bh]
            rt_sb = consts.tile([128, EC, 4 * H], F16, tag="rt")
            for ec in range(EC):
                ps = ps_t.tile([128, 512], F32, tag="pt")
                nc.tensor.matmul(
                    ps[:, :4 * H], r_sb[:, ec * 128:(ec + 1) * 128],
                    i128[:4 * H, :4 * H], start=True, stop=True)
                nc.vector.tensor_copy(out=rt_sb[:, ec, :], in_=ps[:, :4 * H])

            # U^T for all batches, filled in the main loop, consumed in the tail
            ut_all = consts.tile([128, BL, EC, H], F16, tag="ut")

            # ---------------- main loop over local batches ----------------
            # The x-transposes are issued as REGULAR matmuls against an
            # identity rhs (not transpose-mode): transpose-mode runs at the
            # un-HAM'd 1.2 GHz clock (~226 ns/block measured) while a warm
            # regular matmul streams the same block in ~55 ns and keeps the
            # PE's HAM clock at 8/8 for the surrounding matmuls.
            state: dict = {}

            def emit_xdma(b):
                # x arrives in QUARTER tiles (8 s-chunks each): finer slot
                # granularity lets batch b+1's stream begin while U(b) is
                # still consuming the early quarters of batch b.
                src = x_ap[b].rearrange("(sc p) e -> p sc e", p=128)
                xh = []
                for q in range(4):
                    xt_buf = xpool.tile([128, SC // 4, E], F16, tag="xh",
                                        name=f"xh_{b}_{q}")
                    for g in range(2):
                        lo = q * 8 + g * 4
                        nc.gpsimd.dma_start(
                            out=xt_buf[:, g * 4:(g + 1) * 4, :],
                            in_=src[:, lo:lo + 4, :])
                    xh.append(xt_buf)
                state[b] = {
                    "xh": xh,
                    "scores": smpool.tile([H, S], F32, tag="scores",
                                          name=f"scores{b}"),
                    "mx_all": smpool.tile([H, ST], F32, tag="mx_all",
                                          name=f"mxall{b}"),
                }

            def passA_tile(b, t):
                st = state[b]
                xbuf = st["xh"][t // 2]
                sc0 = (t % 2) * 4
                xt = xtpool.tile([128, EC, 512], F16, tag="xt", name=f"xt{b}_{t}")
                for ec in range(EC):
                    ps = ps_tf.tile([128, 512], F32, tag="ptf", name=f"tp{b}_{t}_{ec}")
                    for k in range(4):
                        nc.tensor.matmul(
                            ps[:, k * 128:(k + 1) * 128],
                            xbuf[:, sc0 + k, ec * 128:(ec + 1) * 128],
                            i128[:], start=True, stop=True)
                    if ec % 2 == 0:
                        nc.scalar.copy(out=xt[:, ec, :], in_=ps[:])
                    else:
                        nc.vector.tensor_copy(out=xt[:, ec, :], in_=ps[:])
                sps = ps_s.tile([128, 512], F32, tag="s", name=f"sps{b}_{t}")
                for ec in range(EC):
                    nc.tensor.matmul(
                        sps[:H, :], rt_sb[:, ec, b * H:(b + 1) * H], xt[:, ec, :],
                        start=(ec == 0), stop=(ec == EC - 1))
                mbt = mbpool.tile([H, 512], F16, tag="mb", name=f"mb{b}_{t}")
                nc.sync.dma_start(out=mbt, in_=mb_ap[b, :, t * 512:(t + 1) * 512])
                nc.vector.tensor_tensor(
                    st["scores"][:, t * 512:(t + 1) * 512], sps[:H, :], mbt,
                    mybir.AluOpType.add)
                nc.vector.reduce_max(
                    out=st["mx_all"][:, t:t + 1],
                    in_=st["scores"][:, t * 512:(t + 1) * 512],
                    axis=mybir.AxisListType.X)

            emit_xdma(0)
            wvt_sb = None
            wot_sb = None
            for b in range(BL):
                # batch b+1's x stream: for b=0 it must NOT enter the DMA
                # queue until batch 0's own stream is fully issued (it would
                # steal HBM bandwidth from the critical prologue); later
                # batches have their x resident already, so prefetch freely.
                if 0 < b < BL - 1:
                    emit_xdma(b + 1)
                if b == BL - 1:
                    # prefetch the tail's Wv.T / Wout.T into x-stream slots
                    # via the (now idle) SWDGE queue — the sync queue still
                    # carries this batch's mask tiles.
                    wvt_sb = xpool.tile([128, EC, E], F16, tag="xh",
                                        name="wvt_sb")
                    nc.gpsimd.dma_start(
                        out=wvt_sb,
                        in_=wvt[:].rearrange("(c p) m -> p c m", p=128))
                    wot_sb = xpool.tile([128, EC, E], F16, tag="xh",
                                        name="wot_sb")
                    nc.gpsimd.dma_start(
                        out=wot_sb,
                        in_=wot[:].rearrange("(c p) m -> p c m", p=128))
                st = state[b]
                for t in range(2 if b > 0 else 0, ST):
                    if b == 0:
                        warm(24)
                    passA_tile(b, t)
                if b == 0:
                    emit_xdma(1)
                xh = st["xh"]
                scores_sb = st["scores"]
                mx_all = st["mx_all"]

                # softmax: exp(score - rowmax), accumulate denominator
                p_sb = ppool.tile([H, S], F16, tag="p")
                mx = ppool.tile([H, 1], F32, tag="mx")
                nc.vector.reduce_max(out=mx, in_=mx_all, axis=mybir.AxisListType.X)
                neg_mx = ppool.tile([H, 1], F32, tag="neg_mx")
                nc.scalar.mul(out=neg_mx, in_=mx, mul=-1.0)
                den = ppool.tile([H, 1], F32, tag="den")
                nc.scalar.activation(
                    out=p_sb, in_=scores_sb,
                    func=mybir.ActivationFunctionType.Exp,
                    bias=neg_mx, accum_out=den)
                recip = ppool.tile([H, 1], F32, tag="recip")
                nc.vector.reciprocal(out=recip, in_=den)
                d16 = ppool.tile([H, H], F16, tag="d16")
                nc.vector.tensor_tensor(
                    d16, i16, recip.to_broadcast((H, H)), mybir.AluOpType.mult)
                recip16 = ppool.tile([H, 1], F16, tag="recip16")
                nc.scalar.mul(out=recip16, in_=recip, mul=1.0 / H)

                # keep the PE warm during this batch's softmax chain by
                # pulling the first two pass-A tiles of the next batch forward
                if b + 1 < BL:
                    passA_tile(b + 1, 0)
                    passA_tile(b + 1, 1)

                # p^T (normalized) [s, h] per s-chunk, all in one PSUM bank
                ptT = ptpool.tile([128, SC, H], F16, tag="ptT")
                pps = ps_t.tile([128, 512], F32, tag="pt")
                for sc in range(SC):
                    nc.tensor.matmul(
                        pps[:, sc * H:(sc + 1) * H],
                        p_sb[:, sc * 128:(sc + 1) * 128], d16,
                        start=True, stop=True)
                nc.vector.tensor_copy(out=ptT[:], in_=pps[:])

                # pass B: U[h, e] = p @ x, accumulated over all 32 s-chunks
                ups = [ps_u.tile([128, 512], F32, tag="u", name=f"u{j}") for j in range(2)]
                for sc in range(SC):
                    xbuf = xh[sc // (SC // 4)]
                    scl = sc % (SC // 4)
                    for j in range(2):
                        nc.tensor.matmul(
                            ups[j][:H, :], ptT[:, sc, :],
                            xbuf[:, scl, j * 512:(j + 1) * 512],
                            start=(sc == 0), stop=(sc == SC - 1))
                u_sb = ppool.tile([H, E], F16, tag="u")
                for j in range(2):
                    nc.scalar.copy(
                        out=u_sb[:, j * 512:(j + 1) * 512], in_=ups[j][:H, :])

                # attn_weights = (1/H) * ones @ p  -> [1, S] then DMA out
                for t in range(ST):
                    aps = ps_s.tile([128, 512], F32, tag="s")
                    nc.tensor.matmul(
                        aps[:1, :], recip16, p_sb[:, t * 512:(t + 1) * 512],
                        start=True, stop=True)
                    awt = awpool.tile([1, 512], F32, tag="aw")
                    nc.scalar.copy(out=awt, in_=aps[:1, :])
                    nc.sync.dma_start(
                        out=aw_o[:][b:b + 1, t * 512:(t + 1) * 512], in_=awt)

                # U^T [e, h] for this batch
                utps = ps_t.tile([128, 512], F32, tag="pt")
                for ec in range(EC):
                    nc.tensor.matmul(
                        utps[:, ec * H:(ec + 1) * H],
                        u_sb[:, ec * 128:(ec + 1) * 128], i16,
                        start=True, stop=True)
                nc.vector.tensor_copy(out=ut_all[:, b, :, :], in_=utps[:, :EC * H])

            # ---------------- tail: ctx and output projection ----------------
            # ctx^T[o, b] = sum_e Wv.T[e, o] * U^T[e, h(o)]  (head-blockdiagonal)
            ctxt_sb = consts.tile([128, EC, BL], F16, tag="ctxt")
            for oc in range(EC):
                cps = ps_s.tile([128, 512], F32, tag="s")
                for hh in range(2):
                    h = 2 * oc + hh
                    for ec in range(EC):
                        nc.tensor.matmul(
                            cps[hh * 64:(hh + 1) * 64, :BL],
                            wvt_sb[:, ec, oc * 128 + hh * 64:
                                   oc * 128 + (hh + 1) * 64],
                            ut_all[:, :, ec, h],
                            start=(ec == 0), stop=(ec == EC - 1))
                nc.vector.tensor_tensor(
                    ctxt_sb[:, oc, :], cps[:, :BL],
                    bv_sb[:, oc:oc + 1].to_broadcast((128, BL)),
                    mybir.AluOpType.add)

            # out^T[o2, b] = Wout.T^T-chunks @ ctx^T
            outt_sb = consts.tile([128, EC, BL], F16, tag="outt")
            for o2c in range(EC):
                ops_ = ps_s.tile([128, 512], F32, tag="s")
                for oc in range(EC):
                    nc.tensor.matmul(
                        ops_[:, :BL],
                        wot_sb[:, oc, o2c * 128:(o2c + 1) * 128],
                        ctxt_sb[:, oc, :],
                        start=(oc == 0), stop=(oc == EC - 1))
                nc.scalar.copy(out=outt_sb[:, o2c, :], in_=ops_[:, :BL])

            # transpose out^T back to [b, o2], add out_b, DMA out
            out_sb = consts.tile([BL, E], F32, tag="out_sb")
            for g in range(2):
                onps = ps_t.tile([128, 512], F32, tag="pt")
                for k in range(4):
                    c = g * 4 + k
                    nc.tensor.matmul(
                        onps[:BL, k * 128:(k + 1) * 128],
                        outt_sb[:, c, :], i128[:],
                        start=True, stop=True)
                nc.vector.tensor_tensor(
                    out_sb[:, g * 512:(g + 1) * 512], onps[:BL, :],
                    ob_rep[:, g * 512:(g + 1) * 512], mybir.AluOpType.add)
            nc.sync.dma_start(out=out_o[:], in_=out_sb)

    _split_multiwaits(nc)
    return nc


def _get_nc() -> bass.Bass:
    if "nc" not in _CACHE:
        _CACHE["nc"] = _build_bass()
    return _CACHE["nc"]


def _prep_inputs(x, mask, in_proj_w, in_proj_b, out_w, out_b):
    x = np.asarray(x, dtype=np.float32)
    mask = np.asarray(mask)
    in_proj_w = np.asarray(in_proj_w, dtype=np.float32)
    in_proj_b = np.asarray(in_proj_b, dtype=np.float32)
    out_w = np.asarray(out_w, dtype=np.float32)
    out_b = np.asarray(out_b, dtype=np.float32)

    Wq, Wk, Wv = in_proj_w[:E], in_proj_w[E:2 * E], in_proj_w[2 * E:]
    bq, bv = in_proj_b[:E], in_proj_b[2 * E:]
    sc = 1.0 / np.sqrt(D)

    wqt = np.ascontiguousarray((Wq.T * sc)).astype(np.float16)
    wk16 = np.ascontiguousarray(Wk).astype(np.float16)
    wvt = np.ascontiguousarray(Wv.T).astype(np.float16)
    wot = np.ascontiguousarray(out_w.T).astype(np.float16)
    bq8 = (bq * sc).astype(np.float32)
    bv32 = bv.astype(np.float32)
    ob32 = out_b.astype(np.float32)

    mb = np.where(mask, np.float16(-60000.0), np.float16(0.0)).astype(np.float16)
    mb_rep = np.ascontiguousarray(
        np.broadcast_to(mb[:, None, :], (B, H, S))).astype(np.float16)

    in_maps = []
    for c in range(NCORES):
        lo, hi = c * BL, (c + 1) * BL
        in_maps.append({
            "x_in": np.ascontiguousarray(x[lo:hi]),
            "mb_in": np.ascontiguousarray(mb_rep[lo:hi]),
            "wqt": wqt, "wk": wk16, "wvt": wvt, "wot": wot,
            "bq8": bq8, "bv_d": bv32, "ob_d": ob32,
        })
    return in_maps


def run(trace=False, **inputs):
    nc = _get_nc()
    in_maps = _prep_inputs(**inputs)
    res = run_bass_kernel_spmd(
        nc, in_maps, core_ids=list(range(NCORES)), trace=trace)
    out = np.concatenate([r["out_o"] for r in res.results], axis=0)
    aw = np.concatenate([r["aw_o"] for r in res.results], axis=0)
    out = out.astype(np.float32)
    aw = aw.astype(np.float32)[:, None, :]
    return (out, aw), res


def kernel(**inputs):
    (out, aw), _ = run(trace=False, **inputs)
    return out, aw
